# revision 9
# baseline (speedup 1.0000x reference)
"""3-layer GCNConv on 8 Trainium2 NeuronCores (Bass/Tile).

Sharding: nodes by destination range, 12500 per core.  Per core and layer:
  shard transform (PE) -> AllGather full feature table (4-node-packed 256B
  rows in DRAM) -> per-edge gather (dma_gather, int16 group indices, 4 SWDGE
  queues, 128B payloads) -> 1-of-4 extraction * edge-norm (DVE, static masks)
  -> round-major segmented reduction (each round r holds the r-th in-edge of
  every node, nodes in degree-descending order, so per-round sums are plain
  strided tensor_adds) -> + self-loop + bias -> ELU.
Layer 3 aggregates h3 (4 cols) first and applies W3 afterwards (the linear
transform commutes with the aggregation).  Host numpy does only index/structure
preprocessing; outputs are un-permuted on the host.
"""

import os
import sys

if "/opt/trn_rl_repo" not in sys.path:
    sys.path.insert(0, "/opt/trn_rl_repo")

import numpy as np

import concourse.bass as bass
import concourse.bacc as bacc
import concourse.tile as tile
from concourse import mybir, library_config
from concourse.bass_utils import run_bass_kernel_spmd

N = 100000
F_IN = 256
H1, H2, C_OUT = 8, 4, 16
N_CORES = 8
NPC = N // N_CORES
P = 128
NCOL = NPC // P               # 97.65 -> not integer! NPC=12500 -> 12500/128
# NPC is not a multiple of 128; pad shard rows to 12544 (=98*128)
NROW = ((NPC + P - 1) // P) * P          # 12544 padded shard rows
NCOL = NROW // P                          # 98
NTOT = N_CORES * NROW                     # padded global rows 100352
GROUP = 4
TAB_STRIDE = 64                           # f32 (256B rows)
SUB = 8                                   # f32 sub-slot per node
PAY = GROUP * SUB                         # 32 f32 gathered per index
NGRP = NTOT // GROUP + 1                  # table group rows (+1 zero spare)
NI = 896                                  # idx per gather call
WIN = 16                                  # gather calls per window
N_QUEUES = 4

_F32 = mybir.dt.float32
_I16 = mybir.dt.int16


def _raw_dma_gather(gp, out_ap, in_ap, idxs_ap, num_idxs, elem_size,
                    elem_step, queue_num):
    """nc.gpsimd.dma_gather without the 256B elem_size restriction (sub-256B
    payloads verified against numpy on hardware)."""
    from concourse import ap_utils
    from concourse.bass import exact_div
    assert idxs_ap.dtype == _I16
    assert in_ap.space == bass.MemorySpace.DRAM
    assert out_ap.space == bass.MemorySpace.SBUF
    assert ap_utils.ap_is_contiguous(in_ap.ap[1:])
    assert ap_utils.ap_is_contiguous(out_ap.ap[1:])
    assert num_idxs % 128 == 0
    assert in_ap.ap[-1][1] == out_ap.ap[-1][1] == elem_size
    assert out_ap.ap[0][1] * out_ap.ap[1][1] == num_idxs
    assert in_ap.ap[0][0] == elem_step
    stride_bytes_256 = exact_div(elem_step * 4, 256)
    _in_ap = gp.lower_ap_dma(in_ap, for_custom_bir_dma=True)
    return gp.add_instruction(
        mybir.InstDMAGatherAnt(
            name=gp.bass.get_next_instruction_name(),
            ins=[*_in_ap, gp.lower_ap(idxs_ap),
                 gp.lower_val_access(gp.to_reg(num_idxs))],
            outs=[gp.lower_ap(out_ap)],
            transpose=False, num_idxs=num_idxs, elem_size=elem_size,
            stride_bytes_256=stride_bytes_256, gen_mode=0,
            single_packet=True, queue_num=queue_num,
        ))


def _prep(edge_index, edge_weight):
    src = np.asarray(edge_index[0], np.int64)
    dst = np.asarray(edge_index[1], np.int64)
    w = np.asarray(edge_weight, np.float64)

    deg = np.zeros(N, np.float64)
    np.add.at(deg, dst, w)
    deg += 1.0
    dis = 1.0 / np.sqrt(deg)
    norm_edge = (dis[src] * w * dis[dst]).astype(np.float32)
    norm_self = (dis * dis).astype(np.float32)

    core_of = dst // NPC
    # per-core degree (in-edges only)
    ldeg = np.zeros((N_CORES, NPC), np.int64)
    np.add.at(ldeg, (core_of, dst % NPC), 1)

    # sigma: per core, local nodes sorted by degree descending (stable)
    sigmas = [np.argsort(-ldeg[k], kind="stable") for k in range(N_CORES)]
    # global padded row of node u: core*NROW + rank within sigma
    row_of = np.empty(N, np.int64)
    for k in range(N_CORES):
        row_of[k * NPC + sigmas[k]] = k * NROW + np.arange(NPC)

    # order edges per core by (sigma-rank of dst, arrival) -> rounds
    cores = []
    for k in range(N_CORES):
        sel = np.nonzero(core_of == k)[0]
        rank = row_of[dst[sel]] - k * NROW          # 0..NPC
        order = np.argsort(rank, kind="stable")
        sel = sel[order]
        rank = rank[order]
        degs = ldeg[k][sigmas[k]]                   # descending
        rmax = int(degs.max()) if len(degs) else 0
        # within each node, edge j gets round index 0..deg-1
        starts = np.zeros(NPC + 1, np.int64)
        np.cumsum(degs, out=starts[1:])
        rnd = np.arange(len(sel)) - starts[rank]
        # slots: round-major, round r holds ranks [0, n_r), padded to 128
        n_r = np.array([(degs > r).sum() for r in range(rmax)], np.int64)
        n_r_pad = ((n_r + P - 1) // P) * P
        base = np.zeros(rmax + 1, np.int64)
        np.cumsum(n_r_pad, out=base[1:])
        slots_total = int(base[-1])
        slots_total_pad = ((slots_total + NI - 1) // NI) * NI
        slot_grp = np.full(slots_total_pad, NGRP - 1, np.int16)
        slot_sel = np.zeros(slots_total_pad, np.int64)
        slot_norm = np.zeros(slots_total_pad, np.float32)
        j = base[rnd] + rank
        gr = row_of[src[sel]]
        slot_grp[j] = (gr // GROUP).astype(np.int16)
        slot_sel[j] = gr % GROUP
        slot_norm[j] = norm_edge[sel]
        # masks[g, slot] = norm if sel==g else 0, laid out [g, p, fb]
        fb_total = slots_total_pad // P
        masks = np.zeros((GROUP, slots_total_pad), np.float32)
        masks[slot_sel, np.arange(slots_total_pad)] = slot_norm
        masks = masks.reshape(GROUP, fb_total, P).transpose(2, 0, 1)  # [p,g,fb]
        # idx stream per call: call t covers slots [NI*t, NI*(t+1)),
        # slot j -> int16 entry at [16-part-wrap]: entry i of call at
        # partition i%16, word i//16, replicated on 8 groups of 16 parts
        n_calls = slots_total_pad // NI
        g16 = slot_grp.reshape(n_calls, NI // 16, 16).transpose(0, 2, 1)
        gidx = np.tile(g16, (1, 8, 1)).astype(np.int16)  # [calls,128,NI//16]
        cores.append(dict(
            n_calls=n_calls, fb_total=fb_total, masks=np.ascontiguousarray(masks),
            gidx=np.ascontiguousarray(gidx), n_r=n_r, n_r_pad=n_r_pad,
            base=base, sigma=sigmas[k],
        ))
    return cores, row_of, norm_self


_CACHE = {}
LAST_EXEC_NS = None


def _build(fb_total, n_calls, n_r_list, base_list):
    nc = bacc.Bacc("TRN2", target_bir_lowering=False, debug=False,
                   num_devices=N_CORES, num_swdge_queues=N_QUEUES)

    x_d = nc.dram_tensor("x", [NROW, F_IN], _F32, kind="ExternalInput")
    W1_d = nc.dram_tensor("W1", [F_IN, H1], _F32, kind="ExternalInput")
    W2_d = nc.dram_tensor("W2", [H1, H2], _F32, kind="ExternalInput")
    W3_d = nc.dram_tensor("W3", [H2, C_OUT], _F32, kind="ExternalInput")
    b1_d = nc.dram_tensor("b1", [P, H1], _F32, kind="ExternalInput")
    b2_d = nc.dram_tensor("b2", [P, H2], _F32, kind="ExternalInput")
    b3_d = nc.dram_tensor("b3", [P, C_OUT], _F32, kind="ExternalInput")
    nself_d = nc.dram_tensor("nself", [P, NCOL], _F32, kind="ExternalInput")
    idx_d = nc.dram_tensor("gidx", [n_calls, P, NI // 16], _I16, kind="ExternalInput")
    mask_d = nc.dram_tensor("masks", [P, GROUP, fb_total], _F32, kind="ExternalInput")
    out_d = nc.dram_tensor("out", [P, NCOL * C_OUT], _F32, kind="ExternalOutput")

    tab = nc.dram_tensor("tab", [NGRP, TAB_STRIDE], _F32, kind="Internal")
    cc_in = nc.dram_tensor("cci", [NROW, SUB], _F32, kind="Internal")
    cc_out = nc.dram_tensor("cco", [NTOT, SUB], _F32, kind="Internal",
                            addr_space="Shared")

    FB_W = WIN * (NI // P)
    n_win = (fb_total + FB_W - 1) // FB_W
    LAYER_F = [H1, H2, H2]      # gathered feature widths per layer

    from concourse.masks import make_identity

    with tile.TileContext(nc) as tc:
        with tc.tile_pool(name="pers", bufs=1) as pers, \
             tc.tile_pool(name="work", bufs=2) as work, \
             tc.tile_pool(name="gat", bufs=2) as gat, \
             tc.tile_pool(name="msgp", bufs=1) as msgp, \
             tc.tile_pool(name="accp", bufs=1) as accp, \
             tc.tile_pool(name="psum", bufs=2, space="PSUM") as psum:

            nc.gpsimd.load_library(library_config.mlp)
            ident = pers.tile([P, P], _F32)
            make_identity(nc, ident[:])
            W1_t = pers.tile([P, 2, H1], _F32)
            nc.sync.dma_start(out=W1_t[:],
                              in_=W1_d.ap().rearrange("(a k) h -> k a h", k=P))
            W2_t = pers.tile([H1, H2], _F32)
            nc.sync.dma_start(out=W2_t[:], in_=W2_d.ap())
            W3_t = pers.tile([H2, C_OUT], _F32)
            nc.sync.dma_start(out=W3_t[:], in_=W3_d.ap())
            b_ts = []
            for bd, bw in ((b1_d, H1), (b2_d, H2), (b3_d, C_OUT)):
                bt = pers.tile([P, bw], _F32, tag=f"b{bw}{bd.name}")
                nc.sync.dma_start(out=bt[:], in_=bd.ap())
                b_ts.append(bt)
            nself_t = pers.tile([P, NCOL], _F32)
            nc.sync.dma_start(out=nself_t[:], in_=nself_d.ap())

            h_cur = pers.tile([P, NCOL * SUB], _F32)   # shard features, padded to 8

            def transform_to_table(src_tile, F_l):
                """cc_in <- shard cols 0:SUB of src_tile; AllGather; expand."""
                nc.sync.dma_start(
                    out=cc_in.ap().rearrange("(t p) s -> p t s", p=P),
                    in_=src_tile[:].rearrange("p (t s) -> p t s", s=SUB))
                nc.gpsimd.collective_compute(
                    "AllGather", mybir.AluOpType.bypass,
                    ins=[cc_in.ap()], outs=[cc_out.ap()],
                    replica_groups=[list(range(N_CORES))])
                nc.sync.dma_start(
                    out=tab.ap()[0:NTOT // GROUP, 0:PAY].rearrange(
                        "g (n s) -> g n s", n=GROUP),
                    in_=cc_out.ap().rearrange("(g n) s -> g n s", n=GROUP))

            def conv_layer(li, F_l):
                """Gather+extract+reduce from `tab` into acc [P, NCOL*F_l]."""
                acc = accp.tile([P, NCOL * SUB], _F32, tag=f"acc{li}")
                nc.vector.memset(acc[:], 0.0)
                msg = msgp.tile([P, fb_total * F_l], _F32, tag="msg")
                for wdw in range(n_win):
                    fb0 = wdw * FB_W
                    fbn = min(FB_W, fb_total - fb0)
                    calls = (fbn * P) // NI
                    gbuf = gat.tile([P, FB_W * PAY], _F32, tag="gbuf")
                    for cix in range(calls):
                        call = wdw * WIN + cix
                        it = work.tile([P, NI // 16], _I16, tag="gidx")
                        nc.sync.dma_start(out=it[:], in_=idx_d.ap()[call])
                        fb_c = cix * (NI // P)
                        _raw_dma_gather(
                            nc.gpsimd,
                            gbuf[:].rearrange("p (w y) -> p w y", y=PAY)[
                                :, fb_c:fb_c + NI // P, :],
                            tab.ap()[:, 0:PAY], it[:], NI, PAY, TAB_STRIDE,
                            queue_num=call % N_QUEUES)
                    mks = work.tile([P, GROUP, FB_W], _F32, tag="mks")
                    nc.sync.dma_start(out=mks[:, :, 0:fbn],
                                      in_=mask_d.ap()[:, :, fb0:fb0 + fbn])
                    gv = gbuf[:].rearrange("p (w y) -> p w y", y=PAY)
                    mv = msg[:].rearrange("p (w f) -> p w f", f=F_l)[
                        :, fb0:fb0 + fbn, :]
                    for g in range(GROUP):
                        mk0 = mks[:, g, 0:fbn]
                        mk = bass.AP(mk0.tensor, mk0.offset,
                                     [mk0.ap[0], mk0.ap[1], [0, F_l]])
                        src_g = gv[:, 0:fbn, g * SUB:g * SUB + F_l]
                        if g == 0:
                            nc.vector.tensor_tensor(out=mv, in0=src_g, in1=mk,
                                                    op=mybir.AluOpType.mult)
                        else:
                            t2 = gat.tile([P, FB_W * SUB], _F32, tag="t2")
                            t2v = t2[:].rearrange("p (w f) -> p w f", f=F_l)[
                                :, 0:fbn, :]
                            nc.vector.tensor_tensor(out=t2v, in0=src_g, in1=mk,
                                                    op=mybir.AluOpType.mult)
                            nc.vector.tensor_add(out=mv, in0=mv, in1=t2v)
                # rounds: acc[:, 0:n_r cols] += msg round block
                accv = acc[:].rearrange("p (t s) -> p t s", s=SUB)
                msgv = msg[:].rearrange("p (w f) -> p w f", f=F_l)
                for r, n_r in enumerate(n_r_list):
                    ncols = (int(n_r) + P - 1) // P
                    fb_r = base_list[r] // P
                    nc.vector.tensor_add(
                        out=accv[:, 0:ncols, 0:F_l],
                        in0=accv[:, 0:ncols, 0:F_l],
                        in1=msgv[:, fb_r:fb_r + ncols, :])
                return acc

            def add_self_bias_elu(acc, F_l, bias_t, last=False):
                """acc += nself*h_cur ; += bias ; elu (skip elu if last)."""
                accv = acc[:].rearrange("p (t s) -> p t s", s=SUB)[:, :, 0:F_l]
                hv = h_cur[:].rearrange("p (t s) -> p t s", s=SUB)[:, :, 0:F_l]
                nt = nself_t[:]
                nsv = bass.AP(nt.tensor, nt.offset,
                              [nt.ap[0], nt.ap[1], [0, F_l]])
                sc = work.tile([P, NCOL * SUB], _F32, tag="sc")
                scv = sc[:].rearrange("p (t s) -> p t s", s=SUB)[:, :, 0:F_l]
                nc.vector.tensor_tensor(out=scv, in0=hv, in1=nsv,
                                        op=mybir.AluOpType.mult)
                nc.vector.tensor_add(out=accv, in0=accv, in1=scv)
                bt0 = bias_t[:]
                bv = bass.AP(bt0.tensor, bt0.offset,
                             [bt0.ap[0], [0, NCOL], bt0.ap[1]])
                nc.vector.tensor_add(out=accv, in0=accv, in1=bv)
                if not last:
                    # elu(x) = relu(x) + exp(min(x,0)) - 1
                    t_neg = work.tile([P, NCOL * SUB], _F32, tag="t_neg")
                    tnv = t_neg[:].rearrange("p (t s) -> p t s", s=SUB)[:, :, 0:F_l]
                    nc.vector.tensor_scalar_min(tnv, accv, 0.0)
                    nc.scalar.activation(tnv, tnv,
                                         mybir.ActivationFunctionType.Exp)
                    nc.vector.tensor_scalar_max(accv, accv, 0.0)
                    nc.vector.tensor_add(out=accv, in0=accv, in1=tnv)
                    nc.vector.tensor_scalar_add(accv, accv, -1.0)

            def matmul_shard(dst_tile, src_tile, Wt, F_in_l, F_out_l):
                """dst[:, t*SUB ...] <- (src rows) @ W  via PE transpose."""
                for t in range(NCOL):
                    xT = psum.tile([P, P], _F32, tag="ps_t")
                    nc.tensor.transpose(
                        out=xT[0:F_in_l, :],
                        in_=src_tile[:].rearrange("p (c s) -> p c s", s=SUB)[
                            :, t, 0:F_in_l],
                        identity=ident[:])
                    xTs = work.tile([P, P], _F32, tag="xTs")
                    nc.vector.tensor_copy(out=xTs[0:F_in_l, :], in_=xT[0:F_in_l, :])
                    mp = psum.tile([P, 16], _F32, tag="ps_m")
                    nc.tensor.matmul(mp[:, 0:F_out_l], lhsT=xTs[0:F_in_l, :],
                                     rhs=Wt[:], start=True, stop=True)
                    nc.vector.tensor_copy(
                        out=dst_tile[:].rearrange("p (c s) -> p c s", s=SUB)[
                            :, t, 0:F_out_l],
                        in_=mp[:, 0:F_out_l])

            # ---------------- layer 1: m1 = x @ W1 (full 256-wide transform)
            nc.vector.memset(h_cur[:], 0.0)
            m1 = pers.tile([P, NCOL * SUB], _F32)
            nc.vector.memset(m1[:], 0.0)
            for t in range(NCOL):
                xt = work.tile([P, F_IN], _F32, tag="xt")
                nc.sync.dma_start(out=xt[:], in_=x_d.ap()[t * P:(t + 1) * P, :])
                m1p = psum.tile([P, H1], _F32, tag="ps_m1")
                for half in range(2):
                    xT = psum.tile([P, P], _F32, tag="ps_t")
                    nc.tensor.transpose(out=xT[:],
                                        in_=xt[:, half * P:(half + 1) * P],
                                        identity=ident[:])
                    xTs = work.tile([P, P], _F32, tag="xTs")
                    nc.vector.tensor_copy(out=xTs[:], in_=xT[:])
                    nc.tensor.matmul(m1p[:], lhsT=xTs[:],
                                     rhs=W1_t[:, half, :],
                                     start=(half == 0), stop=(half == 1))
                nc.vector.tensor_copy(
                    out=m1[:].rearrange("p (c s) -> p c s", s=SUB)[:, t, 0:H1],
                    in_=m1p[:])
            # x itself is h for the self-loop of layer 1?  No: layer1 self term
            # uses m1 (aggregation of m1 rows).  h_cur := m1 for self-contrib.
            nc.vector.tensor_copy(out=h_cur[:], in_=m1[:])
            transform_to_table(m1, H1)
            acc1 = conv_layer(0, H1)
            add_self_bias_elu(acc1, H1, b_ts[0])
            # h2 = acc1 (8 cols used)
            nc.vector.memset(h_cur[:], 0.0)
            nc.vector.tensor_copy(
                out=h_cur[:].rearrange("p (c s) -> p c s", s=SUB)[:, :, 0:H1],
                in_=acc1[:].rearrange("p (c s) -> p c s", s=SUB)[:, :, 0:H1])

            # ---------------- layer 2: m2 = h2 @ W2
            m2 = pers.tile([P, NCOL * SUB], _F32, tag="m2")
            nc.vector.memset(m2[:], 0.0)
            matmul_shard(m2, h_cur, W2_t, H1, H2)
            nc.vector.tensor_copy(out=h_cur[:], in_=m2[:])
            transform_to_table(m2, H2)
            acc2 = conv_layer(1, H2)
            add_self_bias_elu(acc2, H2, b_ts[1])
            nc.vector.memset(h_cur[:], 0.0)
            nc.vector.tensor_copy(
                out=h_cur[:].rearrange("p (c s) -> p c s", s=SUB)[:, :, 0:H2],
                in_=acc2[:].rearrange("p (c s) -> p c s", s=SUB)[:, :, 0:H2])

            # ---------------- layer 3: aggregate h3 first, transform after
            transform_to_table(h_cur, H2)
            acc3 = conv_layer(2, H2)
            # self term for aggregation of h3
            accv = acc3[:].rearrange("p (t s) -> p t s", s=SUB)[:, :, 0:H2]
            hv = h_cur[:].rearrange("p (t s) -> p t s", s=SUB)[:, :, 0:H2]
            nt = nself_t[:]
            nsv = bass.AP(nt.tensor, nt.offset, [nt.ap[0], nt.ap[1], [0, H2]])
            sc = work.tile([P, NCOL * SUB], _F32, tag="sc")
            scv = sc[:].rearrange("p (t s) -> p t s", s=SUB)[:, :, 0:H2]
            nc.vector.tensor_tensor(out=scv, in0=hv, in1=nsv,
                                    op=mybir.AluOpType.mult)
            nc.vector.tensor_add(out=accv, in0=accv, in1=scv)
            # out = agg3 @ W3 + b3
            outt = work.tile([P, NCOL * C_OUT], _F32, tag="outt")
            for t in range(NCOL):
                aT = psum.tile([P, P], _F32, tag="ps_t")
                nc.tensor.transpose(
                    out=aT[0:H2, :],
                    in_=acc3[:].rearrange("p (c s) -> p c s", s=SUB)[:, t, 0:H2],
                    identity=ident[:])
                aTs = work.tile([P, P], _F32, tag="xTs")
                nc.vector.tensor_copy(out=aTs[0:H2, :], in_=aT[0:H2, :])
                op = psum.tile([P, 16], _F32, tag="ps_m")
                nc.tensor.matmul(op[:, 0:C_OUT], lhsT=aTs[0:H2, :], rhs=W3_t[:],
                                 start=True, stop=True)
                bv = b_ts[2][:]
                nc.vector.tensor_add(
                    out=outt[:, t * C_OUT:(t + 1) * C_OUT],
                    in0=op[:, 0:C_OUT], in1=bv)
            nc.sync.dma_start(out=out_d.ap(), in_=outt[:])

    nc.compile()
    return nc


def kernel(x, edge_index, edge_weight, W1, b1, W2, b2, W3, b3):
    x = np.asarray(x, np.float32)
    cores, row_of, norm_self = _prep(np.asarray(edge_index),
                                     np.asarray(edge_weight))
    # all cores must share one program: pad structures to common sizes
    fb_total = max(c["fb_total"] for c in cores)
    fb_total = ((fb_total * P + NI - 1) // NI) * NI // P
    n_calls = fb_total * P // NI
    rmax = max(len(c["n_r"]) for c in cores)
    # common padded round bases: use per-core maxima so one program serves all
    n_r_com = np.zeros(rmax, np.int64)
    for c in cores:
        n_r_com[: len(c["n_r"])] = np.maximum(n_r_com[: len(c["n_r"])], c["n_r"])
    n_r_pad = ((n_r_com + P - 1) // P) * P
    base_com = np.zeros(rmax + 1, np.int64)
    np.cumsum(n_r_pad, out=base_com[1:])
    need_fb = int(base_com[-1]) // P
    fb_total = max(fb_total, ((need_fb * P + NI - 1) // NI) * NI // P)
    n_calls = fb_total * P // NI

    # re-layout each core's slots onto the COMMON round bases
    def relayout(k):
        c = cores[k]
        slots = fb_total * P
        grp = np.full(slots, NGRP - 1, np.int16)
        msk = np.zeros((GROUP, slots), np.float32)
        om = c["masks"].transpose(0, 2, 1).reshape(GROUP, -1)  # [g, oldslots]
        for r in range(len(c["n_r"])):
            ob, nb = int(c["base"][r]), int(base_com[r])
            ln = int(c["n_r_pad"][r])
            grp[nb:nb + ln] = np.frombuffer(
                c["gidx"], np.int16).reshape(-1)[0:0] if False else grp[nb:nb + ln]
        # simpler: rebuild from original slot arrays
        return grp, msk

    # Rebuild slot arrays directly on common bases (redo cheap part of prep)
    src = np.asarray(edge_index[0], np.int64)
    dst = np.asarray(edge_index[1], np.int64)
    w64 = np.asarray(edge_weight, np.float64)
    deg = np.zeros(N, np.float64)
    np.add.at(deg, dst, w64)
    deg += 1.0
    dis = 1.0 / np.sqrt(deg)
    norm_edge = (dis[src] * w64 * dis[dst]).astype(np.float32)
    core_of = dst // NPC

    gidx_all, masks_all, nself_all, xs = [], [], [], []
    for k in range(N_CORES):
        c = cores[k]
        sigma = c["sigma"]
        sel = np.nonzero(core_of == k)[0]
        rank_of_local = np.empty(NPC, np.int64)
        rank_of_local[sigma] = np.arange(NPC)
        rank = rank_of_local[dst[sel] - k * NPC]
        order = np.argsort(rank, kind="stable")
        sel = sel[order]
        rank = rank[order]
        degs_sorted = np.bincount(rank, minlength=NPC)
        starts = np.zeros(NPC + 1, np.int64)
        np.cumsum(degs_sorted, out=starts[1:])
        rnd = np.arange(len(sel)) - starts[rank]
        slots = fb_total * P
        grp = np.full(slots, NGRP - 1, np.int16)
        sel4 = np.zeros(slots, np.int64)
        nrm = np.zeros(slots, np.float32)
        j = base_com[rnd] + rank
        gr = row_of[src[sel]]
        grp[j] = (gr // GROUP).astype(np.int16)
        sel4[j] = gr % GROUP
        nrm[j] = norm_edge[sel]
        masks = np.zeros((GROUP, slots), np.float32)
        masks[sel4, np.arange(slots)] = nrm
        masks = masks.reshape(GROUP, fb_total, P).transpose(2, 0, 1)
        g16 = grp.reshape(n_calls, NI // 16, 16).transpose(0, 2, 1)
        gidx_all.append(np.ascontiguousarray(np.tile(g16, (1, 8, 1))))
        masks_all.append(np.ascontiguousarray(masks))
        ns = np.zeros(NROW, np.float32)
        ns[:NPC] = norm_self[k * NPC + sigma]
        nself_all.append(ns.reshape(NCOL, P).T.copy())
        xp = np.zeros((NROW, F_IN), np.float32)
        xp[:NPC] = x[k * NPC + sigma]
        xs.append(xp)

    key = (fb_total, n_calls, rmax)
    if key not in _CACHE:
        _CACHE[key] = _build(fb_total, n_calls,
                             [int(v) for v in n_r_com],
                             [int(v) for v in base_com])
    nc = _CACHE[key]

    ins = []
    for k in range(N_CORES):
        ins.append({
            "x": xs[k],
            "W1": np.asarray(W1, np.float32), "W2": np.asarray(W2, np.float32),
            "W3": np.asarray(W3, np.float32),
            "b1": np.tile(np.asarray(b1, np.float32).reshape(1, H1), (P, 1)),
            "b2": np.tile(np.asarray(b2, np.float32).reshape(1, H2), (P, 1)),
            "b3": np.tile(np.asarray(b3, np.float32).reshape(1, C_OUT), (P, 1)),
            "nself": nself_all[k],
            "gidx": gidx_all[k],
            "masks": masks_all[k],
        })
    trace = bool(os.environ.get("KERNEL_TRACE"))
    res = run_bass_kernel_spmd(nc, ins, core_ids=list(range(N_CORES)),
                               trace=trace)
    global LAST_EXEC_NS
    LAST_EXEC_NS = res.exec_time_ns
    out = np.empty((N, C_OUT), np.float32)
    for k in range(N_CORES):
        o = res.results[k]["out"]            # [P, NCOL*C_OUT]
        rows = o.reshape(P, NCOL, C_OUT).transpose(1, 0, 2).reshape(NROW, C_OUT)
        sigma = cores[k]["sigma"]
        out[k * NPC + sigma] = rows[:NPC]
    return out


# revision 13
# speedup vs baseline: 2.5741x; 2.5741x over previous
"""3-layer GCNConv on 8 Trainium2 NeuronCores (Bass/Tile).

Sharding: nodes by destination range, 12500 per core.  Per core and layer:
  shard transform (PE) -> AllGather full feature table (4-node-packed 256B
  rows in DRAM) -> per-edge gather (dma_gather, int16 group indices, 4 SWDGE
  queues, 128B payloads) -> 1-of-4 extraction * edge-norm (DVE, static masks)
  -> round-major segmented reduction (each round r holds the r-th in-edge of
  every node, nodes in degree-descending order, so per-round sums are plain
  strided tensor_adds) -> + self-loop + bias -> ELU.
Layer 3 aggregates h3 (4 cols) first and applies W3 afterwards (the linear
transform commutes with the aggregation).  Host numpy does only index/structure
preprocessing; outputs are un-permuted on the host.
"""

import os
import sys

if "/opt/trn_rl_repo" not in sys.path:
    sys.path.insert(0, "/opt/trn_rl_repo")

import numpy as np

import concourse.bass as bass
import concourse.bacc as bacc
import concourse.tile as tile
from concourse import mybir, library_config
from concourse.bass_utils import run_bass_kernel_spmd

N = 100000
F_IN = 256
H1, H2, C_OUT = 8, 4, 16
N_CORES = 8
NPC = N // N_CORES
P = 128
NCOL = NPC // P               # 97.65 -> not integer! NPC=12500 -> 12500/128
# NPC is not a multiple of 128; pad shard rows to 12544 (=98*128)
NROW = ((NPC + P - 1) // P) * P          # 12544 padded shard rows
NCOL = NROW // P                          # 98
NTOT = N_CORES * NROW                     # padded global rows 100352
GROUP = 4
TAB_STRIDE = 64                           # f32 (256B rows)
SUB = 8                                   # f32 sub-slot per node
PAY = GROUP * SUB                         # 32 f32 gathered per index
NGRP = NTOT // GROUP + 1                  # table group rows (+1 zero spare)
NI = 896                                  # idx per gather call
WIN = 48                                  # gather calls per window
N_QUEUES = 4

_F32 = mybir.dt.float32
_I16 = mybir.dt.int16


def _raw_dma_gather(gp, out_ap, in_ap, idxs_ap, num_idxs, elem_size,
                    elem_step, queue_num):
    """nc.gpsimd.dma_gather without the 256B elem_size restriction (sub-256B
    payloads verified against numpy on hardware)."""
    from concourse import ap_utils
    from concourse.bass import exact_div
    assert idxs_ap.dtype == _I16
    assert in_ap.space == bass.MemorySpace.DRAM
    assert out_ap.space == bass.MemorySpace.SBUF
    assert ap_utils.ap_is_contiguous(in_ap.ap[1:])
    assert ap_utils.ap_is_contiguous(out_ap.ap[1:])
    assert num_idxs % 128 == 0
    assert in_ap.ap[-1][1] == out_ap.ap[-1][1] == elem_size
    assert out_ap.ap[0][1] * out_ap.ap[1][1] == num_idxs
    assert in_ap.ap[0][0] == elem_step
    stride_bytes_256 = exact_div(elem_step * 4, 256)
    _in_ap = gp.lower_ap_dma(in_ap, for_custom_bir_dma=True)
    return gp.add_instruction(
        mybir.InstDMAGatherAnt(
            name=gp.bass.get_next_instruction_name(),
            ins=[*_in_ap, gp.lower_ap(idxs_ap),
                 gp.lower_val_access(gp.to_reg(num_idxs))],
            outs=[gp.lower_ap(out_ap)],
            transpose=False, num_idxs=num_idxs, elem_size=elem_size,
            stride_bytes_256=stride_bytes_256, gen_mode=0,
            single_packet=True, queue_num=queue_num,
        ))


def _prep(edge_index, edge_weight):
    src = np.asarray(edge_index[0], np.int64)
    dst = np.asarray(edge_index[1], np.int64)
    w = np.asarray(edge_weight, np.float64)

    deg = np.zeros(N, np.float64)
    np.add.at(deg, dst, w)
    deg += 1.0
    dis = 1.0 / np.sqrt(deg)
    norm_edge = (dis[src] * w * dis[dst]).astype(np.float32)
    norm_self = (dis * dis).astype(np.float32)

    core_of = dst // NPC
    # per-core degree (in-edges only)
    ldeg = np.zeros((N_CORES, NPC), np.int64)
    np.add.at(ldeg, (core_of, dst % NPC), 1)

    # sigma: per core, local nodes sorted by degree descending (stable)
    sigmas = [np.argsort(-ldeg[k], kind="stable") for k in range(N_CORES)]
    # global padded row of node u: core*NROW + rank within sigma
    row_of = np.empty(N, np.int64)
    for k in range(N_CORES):
        row_of[k * NPC + sigmas[k]] = k * NROW + np.arange(NPC)

    # order edges per core by (sigma-rank of dst, arrival) -> rounds
    cores = []
    for k in range(N_CORES):
        sel = np.nonzero(core_of == k)[0]
        rank = row_of[dst[sel]] - k * NROW          # 0..NPC
        order = np.argsort(rank, kind="stable")
        sel = sel[order]
        rank = rank[order]
        degs = ldeg[k][sigmas[k]]                   # descending
        rmax = int(degs.max()) if len(degs) else 0
        # within each node, edge j gets round index 0..deg-1
        starts = np.zeros(NPC + 1, np.int64)
        np.cumsum(degs, out=starts[1:])
        rnd = np.arange(len(sel)) - starts[rank]
        # slots: round-major, round r holds ranks [0, n_r), padded to 128
        n_r = np.array([(degs > r).sum() for r in range(rmax)], np.int64)
        n_r_pad = ((n_r + P - 1) // P) * P
        base = np.zeros(rmax + 1, np.int64)
        np.cumsum(n_r_pad, out=base[1:])
        slots_total = int(base[-1])
        slots_total_pad = ((slots_total + NI - 1) // NI) * NI
        slot_grp = np.full(slots_total_pad, NGRP - 1, np.int16)
        slot_sel = np.zeros(slots_total_pad, np.int64)
        slot_norm = np.zeros(slots_total_pad, np.float32)
        j = base[rnd] + rank
        gr = row_of[src[sel]]
        slot_grp[j] = (gr // GROUP).astype(np.int16)
        slot_sel[j] = gr % GROUP
        slot_norm[j] = norm_edge[sel]
        # masks[g, slot] = norm if sel==g else 0, laid out [g, p, fb]
        fb_total = slots_total_pad // P
        masks = np.zeros((GROUP, slots_total_pad), np.float32)
        masks[slot_sel, np.arange(slots_total_pad)] = slot_norm
        masks = masks.reshape(GROUP, fb_total, P).transpose(2, 0, 1)  # [p,g,fb]
        # idx stream per call: call t covers slots [NI*t, NI*(t+1)),
        # slot j -> int16 entry at [16-part-wrap]: entry i of call at
        # partition i%16, word i//16, replicated on 8 groups of 16 parts
        n_calls = slots_total_pad // NI
        g16 = slot_grp.reshape(n_calls, NI // 16, 16).transpose(0, 2, 1)
        gidx = np.tile(g16, (1, 8, 1)).astype(np.int16)  # [calls,128,NI//16]
        cores.append(dict(
            n_calls=n_calls, fb_total=fb_total, masks=np.ascontiguousarray(masks),
            gidx=np.ascontiguousarray(gidx), n_r=n_r, n_r_pad=n_r_pad,
            base=base, sigma=sigmas[k],
        ))
    return cores, row_of, norm_self


_CACHE = {}
LAST_EXEC_NS = None


def _build(fb_total, n_calls, n_r_list, base_list):
    nc = bacc.Bacc("TRN2", target_bir_lowering=False, debug=False,
                   num_devices=N_CORES, num_swdge_queues=N_QUEUES)

    x_d = nc.dram_tensor("x", [NROW, F_IN], _F32, kind="ExternalInput")
    W1_d = nc.dram_tensor("W1", [F_IN, H1], _F32, kind="ExternalInput")
    W2_d = nc.dram_tensor("W2", [H1, H2], _F32, kind="ExternalInput")
    W3_d = nc.dram_tensor("W3", [H2, C_OUT], _F32, kind="ExternalInput")
    b1_d = nc.dram_tensor("b1", [P, H1], _F32, kind="ExternalInput")
    b2_d = nc.dram_tensor("b2", [P, H2], _F32, kind="ExternalInput")
    b3_d = nc.dram_tensor("b3", [P, C_OUT], _F32, kind="ExternalInput")
    nself_d = nc.dram_tensor("nself", [P, NCOL], _F32, kind="ExternalInput")
    idx_d = nc.dram_tensor("gidx", [n_calls, P, NI // 16], _I16, kind="ExternalInput")
    mask_d = nc.dram_tensor("masks", [P, GROUP, fb_total], _F32, kind="ExternalInput")
    out_d = nc.dram_tensor("out", [P, NCOL * C_OUT], _F32, kind="ExternalOutput")

    tab = nc.dram_tensor("tab", [NGRP, TAB_STRIDE], _F32, kind="Internal")
    cc_in = nc.dram_tensor("cci", [NROW, SUB], _F32, kind="Internal")
    cc_out = nc.dram_tensor("cco", [NTOT, SUB], _F32, kind="Internal",
                            addr_space="Shared")

    FB_W = WIN * (NI // P)
    n_win = (fb_total + FB_W - 1) // FB_W
    LAYER_F = [H1, H2, H2]      # gathered feature widths per layer

    from concourse.masks import make_identity

    with tile.TileContext(nc) as tc:
        with tc.tile_pool(name="pers", bufs=1) as pers, \
             tc.tile_pool(name="work", bufs=2) as work, \
             tc.tile_pool(name="gat", bufs=2) as gat, \
             tc.tile_pool(name="accp", bufs=1) as accp, \
             tc.tile_pool(name="psum", bufs=2, space="PSUM") as psum:

            nc.gpsimd.load_library(library_config.mlp)
            ident = pers.tile([P, P], _F32)
            make_identity(nc, ident[:])
            W1_t = pers.tile([P, 2, H1], _F32)
            nc.sync.dma_start(out=W1_t[:],
                              in_=W1_d.ap().rearrange("(a k) h -> k a h", k=P))
            W2_t = pers.tile([H1, H2], _F32)
            nc.sync.dma_start(out=W2_t[:], in_=W2_d.ap())
            W3_t = pers.tile([H2, C_OUT], _F32)
            nc.sync.dma_start(out=W3_t[:], in_=W3_d.ap())
            b_ts = []
            for bd, bw in ((b1_d, H1), (b2_d, H2), (b3_d, C_OUT)):
                bt = pers.tile([P, bw], _F32, tag=f"b{bw}{bd.name}")
                nc.sync.dma_start(out=bt[:], in_=bd.ap())
                b_ts.append(bt)
            nself_t = pers.tile([P, NCOL], _F32)
            nc.sync.dma_start(out=nself_t[:], in_=nself_d.ap())

            h_cur = pers.tile([P, NCOL * SUB], _F32)   # shard features, padded to 8

            def transform_to_table(src_tile, F_l):
                """cc_in <- shard cols 0:SUB of src_tile; AllGather; expand."""
                nc.sync.dma_start(
                    out=cc_in.ap().rearrange("(t p) s -> p t s", p=P),
                    in_=src_tile[:].rearrange("p (t s) -> p t s", s=SUB))
                nc.gpsimd.collective_compute(
                    "AllGather", mybir.AluOpType.bypass,
                    ins=[cc_in.ap()], outs=[cc_out.ap()],
                    replica_groups=[list(range(N_CORES))])
                nc.sync.dma_start(
                    out=tab.ap()[0:NTOT // GROUP, 0:PAY].rearrange(
                        "g (n s) -> g n s", n=GROUP),
                    in_=cc_out.ap().rearrange("(g n) s -> g n s", n=GROUP))

            def conv_layer(li, F_l):
                """Gather+extract+reduce from `tab` into acc [P, NCOL*F_l]."""
                acc = accp.tile([P, NCOL * SUB], _F32, tag=f"acc{li}")
                nc.vector.memset(acc[:], 0.0)
                for wdw in range(n_win):
                    fb0 = wdw * FB_W
                    fbn = min(FB_W, fb_total - fb0)
                    calls = (fbn * P) // NI
                    gbuf = gat.tile([P, FB_W * PAY], _F32, tag="gbuf")
                    it = work.tile([P, WIN, NI // 16], _I16, tag="gidx")
                    nc.sync.dma_start(
                        out=it[:, 0:calls, :],
                        in_=idx_d.ap()[wdw * WIN:wdw * WIN + calls].rearrange(
                            "c p w -> p c w"))
                    for cix in range(calls):
                        call = wdw * WIN + cix
                        fb_c = cix * (NI // P)
                        _raw_dma_gather(
                            nc.gpsimd,
                            gbuf[:].rearrange("p (w y) -> p w y", y=PAY)[
                                :, fb_c:fb_c + NI // P, :],
                            tab.ap()[:, 0:PAY], it[:, cix, :], NI, PAY, TAB_STRIDE,
                            queue_num=call % N_QUEUES)
                    mks = work.tile([P, GROUP, FB_W], _F32, tag="mks")
                    nc.sync.dma_start(out=mks[:, :, 0:fbn],
                                      in_=mask_d.ap()[:, :, fb0:fb0 + fbn])
                    gv = gbuf[:].rearrange("p (w y) -> p w y", y=PAY)
                    msg = gat.tile([P, FB_W * SUB], _F32, tag="msgw")
                    mv = msg[:].rearrange("p (w f) -> p w f", f=F_l)[:, 0:fbn, :]
                    for g in range(GROUP):
                        mk0 = mks[:, g, 0:fbn]
                        mk = bass.AP(mk0.tensor, mk0.offset,
                                     [mk0.ap[0], mk0.ap[1], [0, F_l]])
                        src_g = gv[:, 0:fbn, g * SUB:g * SUB + F_l]
                        if g == 0:
                            nc.vector.tensor_tensor(out=mv, in0=src_g, in1=mk,
                                                    op=mybir.AluOpType.mult)
                        else:
                            t2 = gat.tile([P, FB_W * SUB], _F32, tag="t2")
                            t2v = t2[:].rearrange("p (w f) -> p w f", f=F_l)[
                                :, 0:fbn, :]
                            nc.vector.tensor_tensor(out=t2v, in0=src_g, in1=mk,
                                                    op=mybir.AluOpType.mult)
                            nc.vector.tensor_add(out=mv, in0=mv, in1=t2v)
                    # rounds intersecting this window: partial strided adds
                    accv = acc[:].rearrange("p (t s) -> p t s", s=SUB)
                    msgv = msg[:].rearrange("p (w f) -> p w f", f=F_l)
                    for r, n_r in enumerate(n_r_list):
                        ncols = (int(n_r) + P - 1) // P
                        rb = base_list[r] // P
                        lo = max(rb, fb0)
                        hi = min(rb + ncols, fb0 + fbn)
                        if lo >= hi:
                            continue
                        nc.vector.tensor_add(
                            out=accv[:, lo - rb:hi - rb, 0:F_l],
                            in0=accv[:, lo - rb:hi - rb, 0:F_l],
                            in1=msgv[:, lo - fb0:hi - fb0, :])
                return acc

            def add_self_bias_elu(acc, F_l, bias_t, last=False):
                """acc += nself*h_cur ; += bias ; elu (skip elu if last)."""
                accv = acc[:].rearrange("p (t s) -> p t s", s=SUB)[:, :, 0:F_l]
                hv = h_cur[:].rearrange("p (t s) -> p t s", s=SUB)[:, :, 0:F_l]
                nt = nself_t[:]
                nsv = bass.AP(nt.tensor, nt.offset,
                              [nt.ap[0], nt.ap[1], [0, F_l]])
                sc = work.tile([P, NCOL * SUB], _F32, tag="sc")
                scv = sc[:].rearrange("p (t s) -> p t s", s=SUB)[:, :, 0:F_l]
                nc.vector.tensor_tensor(out=scv, in0=hv, in1=nsv,
                                        op=mybir.AluOpType.mult)
                nc.vector.tensor_add(out=accv, in0=accv, in1=scv)
                bt0 = bias_t[:]
                bv = bass.AP(bt0.tensor, bt0.offset,
                             [bt0.ap[0], [0, NCOL], bt0.ap[1]])
                nc.vector.tensor_add(out=accv, in0=accv, in1=bv)
                if not last:
                    # elu(x) = relu(x) + exp(min(x,0)) - 1
                    t_neg = work.tile([P, NCOL * SUB], _F32, tag="t_neg")
                    tnv = t_neg[:].rearrange("p (t s) -> p t s", s=SUB)[:, :, 0:F_l]
                    nc.vector.tensor_scalar_min(tnv, accv, 0.0)
                    nc.scalar.activation(tnv, tnv,
                                         mybir.ActivationFunctionType.Exp)
                    nc.vector.tensor_scalar_max(accv, accv, 0.0)
                    nc.vector.tensor_add(out=accv, in0=accv, in1=tnv)
                    nc.vector.tensor_scalar_add(accv, accv, -1.0)

            def matmul_shard(dst_tile, src_tile, Wt, F_in_l, F_out_l):
                """dst[:, t*SUB ...] <- (src rows) @ W  via PE transpose."""
                for t in range(NCOL):
                    xT = psum.tile([P, P], _F32, tag="ps_t")
                    nc.tensor.transpose(
                        out=xT[0:F_in_l, :],
                        in_=src_tile[:].rearrange("p (c s) -> p c s", s=SUB)[
                            :, t, 0:F_in_l],
                        identity=ident[:])
                    xTs = work.tile([P, P], _F32, tag="xTs")
                    nc.vector.tensor_copy(out=xTs[0:F_in_l, :], in_=xT[0:F_in_l, :])
                    mp = psum.tile([P, 16], _F32, tag="ps_m")
                    nc.tensor.matmul(mp[:, 0:F_out_l], lhsT=xTs[0:F_in_l, :],
                                     rhs=Wt[:], start=True, stop=True)
                    nc.vector.tensor_copy(
                        out=dst_tile[:].rearrange("p (c s) -> p c s", s=SUB)[
                            :, t, 0:F_out_l],
                        in_=mp[:, 0:F_out_l])

            # ---------------- layer 1: m1 = x @ W1 (full 256-wide transform)
            nc.vector.memset(h_cur[:], 0.0)
            m1 = pers.tile([P, NCOL * SUB], _F32)
            nc.vector.memset(m1[:], 0.0)
            for t in range(NCOL):
                xt = work.tile([P, F_IN], _F32, tag="xt")
                nc.sync.dma_start(out=xt[:], in_=x_d.ap()[t * P:(t + 1) * P, :])
                m1p = psum.tile([P, H1], _F32, tag="ps_m1")
                for half in range(2):
                    xT = psum.tile([P, P], _F32, tag="ps_t")
                    nc.tensor.transpose(out=xT[:],
                                        in_=xt[:, half * P:(half + 1) * P],
                                        identity=ident[:])
                    xTs = work.tile([P, P], _F32, tag="xTs")
                    nc.vector.tensor_copy(out=xTs[:], in_=xT[:])
                    nc.tensor.matmul(m1p[:], lhsT=xTs[:],
                                     rhs=W1_t[:, half, :],
                                     start=(half == 0), stop=(half == 1))
                nc.vector.tensor_copy(
                    out=m1[:].rearrange("p (c s) -> p c s", s=SUB)[:, t, 0:H1],
                    in_=m1p[:])
            # x itself is h for the self-loop of layer 1?  No: layer1 self term
            # uses m1 (aggregation of m1 rows).  h_cur := m1 for self-contrib.
            nc.vector.tensor_copy(out=h_cur[:], in_=m1[:])
            transform_to_table(m1, H1)
            acc1 = conv_layer(0, H1)
            add_self_bias_elu(acc1, H1, b_ts[0])
            # h2 = acc1 (8 cols used)
            nc.vector.memset(h_cur[:], 0.0)
            nc.vector.tensor_copy(
                out=h_cur[:].rearrange("p (c s) -> p c s", s=SUB)[:, :, 0:H1],
                in_=acc1[:].rearrange("p (c s) -> p c s", s=SUB)[:, :, 0:H1])

            # ---------------- layer 2: m2 = h2 @ W2
            m2 = pers.tile([P, NCOL * SUB], _F32, tag="m2")
            nc.vector.memset(m2[:], 0.0)
            matmul_shard(m2, h_cur, W2_t, H1, H2)
            nc.vector.tensor_copy(out=h_cur[:], in_=m2[:])
            transform_to_table(m2, H2)
            acc2 = conv_layer(1, H2)
            add_self_bias_elu(acc2, H2, b_ts[1])
            nc.vector.memset(h_cur[:], 0.0)
            nc.vector.tensor_copy(
                out=h_cur[:].rearrange("p (c s) -> p c s", s=SUB)[:, :, 0:H2],
                in_=acc2[:].rearrange("p (c s) -> p c s", s=SUB)[:, :, 0:H2])

            # ---------------- layer 3: aggregate h3 first, transform after
            transform_to_table(h_cur, H2)
            acc3 = conv_layer(2, H2)
            # self term for aggregation of h3
            accv = acc3[:].rearrange("p (t s) -> p t s", s=SUB)[:, :, 0:H2]
            hv = h_cur[:].rearrange("p (t s) -> p t s", s=SUB)[:, :, 0:H2]
            nt = nself_t[:]
            nsv = bass.AP(nt.tensor, nt.offset, [nt.ap[0], nt.ap[1], [0, H2]])
            sc = work.tile([P, NCOL * SUB], _F32, tag="sc")
            scv = sc[:].rearrange("p (t s) -> p t s", s=SUB)[:, :, 0:H2]
            nc.vector.tensor_tensor(out=scv, in0=hv, in1=nsv,
                                    op=mybir.AluOpType.mult)
            nc.vector.tensor_add(out=accv, in0=accv, in1=scv)
            # out = agg3 @ W3 + b3
            outt = work.tile([P, NCOL * C_OUT], _F32, tag="outt")
            for t in range(NCOL):
                aT = psum.tile([P, P], _F32, tag="ps_t")
                nc.tensor.transpose(
                    out=aT[0:H2, :],
                    in_=acc3[:].rearrange("p (c s) -> p c s", s=SUB)[:, t, 0:H2],
                    identity=ident[:])
                aTs = work.tile([P, P], _F32, tag="xTs")
                nc.vector.tensor_copy(out=aTs[0:H2, :], in_=aT[0:H2, :])
                op = psum.tile([P, 16], _F32, tag="ps_m")
                nc.tensor.matmul(op[:, 0:C_OUT], lhsT=aTs[0:H2, :], rhs=W3_t[:],
                                 start=True, stop=True)
                bv = b_ts[2][:]
                nc.vector.tensor_add(
                    out=outt[:, t * C_OUT:(t + 1) * C_OUT],
                    in0=op[:, 0:C_OUT], in1=bv)
            nc.sync.dma_start(out=out_d.ap(), in_=outt[:])

    nc.compile()
    return nc


def kernel(x, edge_index, edge_weight, W1, b1, W2, b2, W3, b3):
    x = np.asarray(x, np.float32)
    cores, row_of, norm_self = _prep(np.asarray(edge_index),
                                     np.asarray(edge_weight))
    # all cores must share one program: pad structures to common sizes
    fb_total = max(c["fb_total"] for c in cores)
    fb_total = ((fb_total * P + NI - 1) // NI) * NI // P
    n_calls = fb_total * P // NI
    rmax = max(len(c["n_r"]) for c in cores)
    # common padded round bases: use per-core maxima so one program serves all
    n_r_com = np.zeros(rmax, np.int64)
    for c in cores:
        n_r_com[: len(c["n_r"])] = np.maximum(n_r_com[: len(c["n_r"])], c["n_r"])
    n_r_pad = ((n_r_com + P - 1) // P) * P
    base_com = np.zeros(rmax + 1, np.int64)
    np.cumsum(n_r_pad, out=base_com[1:])
    need_fb = int(base_com[-1]) // P
    fb_total = max(fb_total, ((need_fb * P + NI - 1) // NI) * NI // P)
    n_calls = fb_total * P // NI

    # re-layout each core's slots onto the COMMON round bases
    def relayout(k):
        c = cores[k]
        slots = fb_total * P
        grp = np.full(slots, NGRP - 1, np.int16)
        msk = np.zeros((GROUP, slots), np.float32)
        om = c["masks"].transpose(0, 2, 1).reshape(GROUP, -1)  # [g, oldslots]
        for r in range(len(c["n_r"])):
            ob, nb = int(c["base"][r]), int(base_com[r])
            ln = int(c["n_r_pad"][r])
            grp[nb:nb + ln] = np.frombuffer(
                c["gidx"], np.int16).reshape(-1)[0:0] if False else grp[nb:nb + ln]
        # simpler: rebuild from original slot arrays
        return grp, msk

    # Rebuild slot arrays directly on common bases (redo cheap part of prep)
    src = np.asarray(edge_index[0], np.int64)
    dst = np.asarray(edge_index[1], np.int64)
    w64 = np.asarray(edge_weight, np.float64)
    deg = np.zeros(N, np.float64)
    np.add.at(deg, dst, w64)
    deg += 1.0
    dis = 1.0 / np.sqrt(deg)
    norm_edge = (dis[src] * w64 * dis[dst]).astype(np.float32)
    core_of = dst // NPC

    gidx_all, masks_all, nself_all, xs = [], [], [], []
    for k in range(N_CORES):
        c = cores[k]
        sigma = c["sigma"]
        sel = np.nonzero(core_of == k)[0]
        rank_of_local = np.empty(NPC, np.int64)
        rank_of_local[sigma] = np.arange(NPC)
        rank = rank_of_local[dst[sel] - k * NPC]
        order = np.argsort(rank, kind="stable")
        sel = sel[order]
        rank = rank[order]
        degs_sorted = np.bincount(rank, minlength=NPC)
        starts = np.zeros(NPC + 1, np.int64)
        np.cumsum(degs_sorted, out=starts[1:])
        rnd = np.arange(len(sel)) - starts[rank]
        slots = fb_total * P
        grp = np.full(slots, NGRP - 1, np.int16)
        sel4 = np.zeros(slots, np.int64)
        nrm = np.zeros(slots, np.float32)
        j = base_com[rnd] + rank
        gr = row_of[src[sel]]
        grp[j] = (gr // GROUP).astype(np.int16)
        sel4[j] = gr % GROUP
        nrm[j] = norm_edge[sel]
        masks = np.zeros((GROUP, slots), np.float32)
        masks[sel4, np.arange(slots)] = nrm
        masks = masks.reshape(GROUP, fb_total, P).transpose(2, 0, 1)
        g16 = grp.reshape(n_calls, NI // 16, 16).transpose(0, 2, 1)
        gidx_all.append(np.ascontiguousarray(np.tile(g16, (1, 8, 1))))
        masks_all.append(np.ascontiguousarray(masks))
        ns = np.zeros(NROW, np.float32)
        ns[:NPC] = norm_self[k * NPC + sigma]
        nself_all.append(ns.reshape(NCOL, P).T.copy())
        xp = np.zeros((NROW, F_IN), np.float32)
        xp[:NPC] = x[k * NPC + sigma]
        xs.append(xp)

    key = (fb_total, n_calls, rmax)
    if key not in _CACHE:
        _CACHE[key] = _build(fb_total, n_calls,
                             [int(v) for v in n_r_com],
                             [int(v) for v in base_com])
    nc = _CACHE[key]

    ins = []
    for k in range(N_CORES):
        ins.append({
            "x": xs[k],
            "W1": np.asarray(W1, np.float32), "W2": np.asarray(W2, np.float32),
            "W3": np.asarray(W3, np.float32),
            "b1": np.tile(np.asarray(b1, np.float32).reshape(1, H1), (P, 1)),
            "b2": np.tile(np.asarray(b2, np.float32).reshape(1, H2), (P, 1)),
            "b3": np.tile(np.asarray(b3, np.float32).reshape(1, C_OUT), (P, 1)),
            "nself": nself_all[k],
            "gidx": gidx_all[k],
            "masks": masks_all[k],
        })
    trace = bool(os.environ.get("KERNEL_TRACE"))
    res = run_bass_kernel_spmd(nc, ins, core_ids=list(range(N_CORES)),
                               trace=trace)
    global LAST_EXEC_NS
    LAST_EXEC_NS = res.exec_time_ns
    out = np.empty((N, C_OUT), np.float32)
    for k in range(N_CORES):
        o = res.results[k]["out"]            # [P, NCOL*C_OUT]
        rows = o.reshape(P, NCOL, C_OUT).transpose(1, 0, 2).reshape(NROW, C_OUT)
        sigma = cores[k]["sigma"]
        out[k * NPC + sigma] = rows[:NPC]
    return out


# revision 15
# speedup vs baseline: 2.5789x; 1.0019x over previous
"""3-layer GCNConv on 8 Trainium2 NeuronCores (Bass/Tile).

Sharding: nodes by destination range, 12500 per core.  Per core and layer:
  shard transform (PE) -> AllGather full feature table (4-node-packed 256B
  rows in DRAM) -> per-edge gather (dma_gather, int16 group indices, 4 SWDGE
  queues, 128B payloads) -> 1-of-4 extraction * edge-norm (DVE, static masks)
  -> round-major segmented reduction (each round r holds the r-th in-edge of
  every node, nodes in degree-descending order, so per-round sums are plain
  strided tensor_adds) -> + self-loop + bias -> ELU.
Layer 3 aggregates h3 (4 cols) first and applies W3 afterwards (the linear
transform commutes with the aggregation).  Host numpy does only index/structure
preprocessing; outputs are un-permuted on the host.
"""

import os
import sys

if "/opt/trn_rl_repo" not in sys.path:
    sys.path.insert(0, "/opt/trn_rl_repo")

import numpy as np

import concourse.bass as bass
import concourse.bacc as bacc
import concourse.tile as tile
from concourse import mybir, library_config
from concourse.bass_utils import run_bass_kernel_spmd

N = 100000
F_IN = 256
H1, H2, C_OUT = 8, 4, 16
N_CORES = 8
NPC = N // N_CORES
P = 128
NCOL = NPC // P               # 97.65 -> not integer! NPC=12500 -> 12500/128
# NPC is not a multiple of 128; pad shard rows to 12544 (=98*128)
NROW = ((NPC + P - 1) // P) * P          # 12544 padded shard rows
NCOL = NROW // P                          # 98
NTOT = N_CORES * NROW                     # padded global rows 100352
GROUP = 4
TAB_STRIDE = 64                           # f32 (256B rows)
SUB = 8                                   # f32 sub-slot per node
PAY = GROUP * SUB                         # 32 f32 gathered per index
NGRP = NTOT // GROUP + 1                  # table group rows (+1 zero spare)
NI = 896                                  # idx per gather call
WIN = 40                                  # gather calls per window
N_QUEUES = 4

_F32 = mybir.dt.float32
_I16 = mybir.dt.int16


def _raw_dma_gather(gp, out_ap, in_ap, idxs_ap, num_idxs, elem_size,
                    elem_step, queue_num):
    """nc.gpsimd.dma_gather without the 256B elem_size restriction (sub-256B
    payloads verified against numpy on hardware)."""
    from concourse import ap_utils
    from concourse.bass import exact_div
    assert idxs_ap.dtype == _I16
    assert in_ap.space == bass.MemorySpace.DRAM
    assert out_ap.space == bass.MemorySpace.SBUF
    assert ap_utils.ap_is_contiguous(in_ap.ap[1:])
    assert ap_utils.ap_is_contiguous(out_ap.ap[1:])
    assert num_idxs % 128 == 0
    assert in_ap.ap[-1][1] == out_ap.ap[-1][1] == elem_size
    assert out_ap.ap[0][1] * out_ap.ap[1][1] == num_idxs
    assert in_ap.ap[0][0] == elem_step
    stride_bytes_256 = exact_div(elem_step * 4, 256)
    _in_ap = gp.lower_ap_dma(in_ap, for_custom_bir_dma=True)
    return gp.add_instruction(
        mybir.InstDMAGatherAnt(
            name=gp.bass.get_next_instruction_name(),
            ins=[*_in_ap, gp.lower_ap(idxs_ap),
                 gp.lower_val_access(gp.to_reg(num_idxs))],
            outs=[gp.lower_ap(out_ap)],
            transpose=False, num_idxs=num_idxs, elem_size=elem_size,
            stride_bytes_256=stride_bytes_256, gen_mode=0,
            single_packet=True, queue_num=queue_num,
        ))


def _prep(edge_index, edge_weight):
    src = np.asarray(edge_index[0], np.int64)
    dst = np.asarray(edge_index[1], np.int64)
    w = np.asarray(edge_weight, np.float64)

    deg = np.zeros(N, np.float64)
    np.add.at(deg, dst, w)
    deg += 1.0
    dis = 1.0 / np.sqrt(deg)
    norm_edge = (dis[src] * w * dis[dst]).astype(np.float32)
    norm_self = (dis * dis).astype(np.float32)

    core_of = dst // NPC
    # per-core degree (in-edges only)
    ldeg = np.zeros((N_CORES, NPC), np.int64)
    np.add.at(ldeg, (core_of, dst % NPC), 1)

    # sigma: per core, local nodes sorted by degree descending (stable)
    sigmas = [np.argsort(-ldeg[k], kind="stable") for k in range(N_CORES)]
    # global padded row of node u: core*NROW + rank within sigma
    row_of = np.empty(N, np.int64)
    for k in range(N_CORES):
        row_of[k * NPC + sigmas[k]] = k * NROW + np.arange(NPC)

    # order edges per core by (sigma-rank of dst, arrival) -> rounds
    cores = []
    for k in range(N_CORES):
        sel = np.nonzero(core_of == k)[0]
        rank = row_of[dst[sel]] - k * NROW          # 0..NPC
        order = np.argsort(rank, kind="stable")
        sel = sel[order]
        rank = rank[order]
        degs = ldeg[k][sigmas[k]]                   # descending
        rmax = int(degs.max()) if len(degs) else 0
        # within each node, edge j gets round index 0..deg-1
        starts = np.zeros(NPC + 1, np.int64)
        np.cumsum(degs, out=starts[1:])
        rnd = np.arange(len(sel)) - starts[rank]
        # slots: round-major, round r holds ranks [0, n_r), padded to 128
        n_r = np.array([(degs > r).sum() for r in range(rmax)], np.int64)
        n_r_pad = ((n_r + P - 1) // P) * P
        base = np.zeros(rmax + 1, np.int64)
        np.cumsum(n_r_pad, out=base[1:])
        slots_total = int(base[-1])
        slots_total_pad = ((slots_total + NI - 1) // NI) * NI
        slot_grp = np.full(slots_total_pad, NGRP - 1, np.int16)
        slot_sel = np.zeros(slots_total_pad, np.int64)
        slot_norm = np.zeros(slots_total_pad, np.float32)
        j = base[rnd] + rank
        gr = row_of[src[sel]]
        slot_grp[j] = (gr // GROUP).astype(np.int16)
        slot_sel[j] = gr % GROUP
        slot_norm[j] = norm_edge[sel]
        # masks[g, slot] = norm if sel==g else 0, laid out [g, p, fb]
        fb_total = slots_total_pad // P
        masks = np.zeros((GROUP, slots_total_pad), np.float32)
        masks[slot_sel, np.arange(slots_total_pad)] = slot_norm
        masks = masks.reshape(GROUP, fb_total, P).transpose(2, 0, 1)  # [p,g,fb]
        # idx stream per call: call t covers slots [NI*t, NI*(t+1)),
        # slot j -> int16 entry at [16-part-wrap]: entry i of call at
        # partition i%16, word i//16, replicated on 8 groups of 16 parts
        n_calls = slots_total_pad // NI
        g16 = slot_grp.reshape(n_calls, NI // 16, 16).transpose(0, 2, 1)
        gidx = np.tile(g16, (1, 8, 1)).astype(np.int16)  # [calls,128,NI//16]
        cores.append(dict(
            n_calls=n_calls, fb_total=fb_total, masks=np.ascontiguousarray(masks),
            gidx=np.ascontiguousarray(gidx), n_r=n_r, n_r_pad=n_r_pad,
            base=base, sigma=sigmas[k],
        ))
    return cores, row_of, norm_self


_CACHE = {}
LAST_EXEC_NS = None


def _build(fb_total, n_calls, n_r_list, base_list):
    nc = bacc.Bacc("TRN2", target_bir_lowering=False, debug=False,
                   num_devices=N_CORES, num_swdge_queues=N_QUEUES,
                   dynamic_dma_scratch_size=32768)

    x_d = nc.dram_tensor("x", [NROW, F_IN], _F32, kind="ExternalInput")
    W1_d = nc.dram_tensor("W1", [F_IN, H1], _F32, kind="ExternalInput")
    W2_d = nc.dram_tensor("W2", [H1, H2], _F32, kind="ExternalInput")
    W3_d = nc.dram_tensor("W3", [H2, C_OUT], _F32, kind="ExternalInput")
    b1_d = nc.dram_tensor("b1", [P, H1], _F32, kind="ExternalInput")
    b2_d = nc.dram_tensor("b2", [P, H2], _F32, kind="ExternalInput")
    b3_d = nc.dram_tensor("b3", [P, C_OUT], _F32, kind="ExternalInput")
    nself_d = nc.dram_tensor("nself", [P, NCOL], _F32, kind="ExternalInput")
    idx_d = nc.dram_tensor("gidx", [n_calls, P, NI // 16], _I16, kind="ExternalInput")
    mask_d = nc.dram_tensor("masks", [P, GROUP, fb_total], _F32, kind="ExternalInput")
    out_d = nc.dram_tensor("out", [P, NCOL * C_OUT], _F32, kind="ExternalOutput")

    tab = nc.dram_tensor("tab", [NGRP, TAB_STRIDE], _F32, kind="Internal")
    cc_in = nc.dram_tensor("cci", [NROW, SUB], _F32, kind="Internal")
    cc_out = nc.dram_tensor("cco", [NTOT, SUB], _F32, kind="Internal",
                            addr_space="Shared")

    FB_W = WIN * (NI // P)
    n_win = (fb_total + FB_W - 1) // FB_W
    LAYER_F = [H1, H2, H2]      # gathered feature widths per layer

    from concourse.masks import make_identity

    with tile.TileContext(nc) as tc:
        with tc.tile_pool(name="pers", bufs=1) as pers, \
             tc.tile_pool(name="work", bufs=2) as work, \
             tc.tile_pool(name="gat", bufs=2) as gat, \
             tc.tile_pool(name="accp", bufs=1) as accp, \
             tc.tile_pool(name="psum", bufs=2, space="PSUM") as psum:

            nc.gpsimd.load_library(library_config.mlp)
            ident = pers.tile([P, P], _F32)
            make_identity(nc, ident[:])
            W1_t = pers.tile([P, 2, H1], _F32)
            nc.sync.dma_start(out=W1_t[:],
                              in_=W1_d.ap().rearrange("(a k) h -> k a h", k=P))
            W2_t = pers.tile([H1, H2], _F32)
            nc.sync.dma_start(out=W2_t[:], in_=W2_d.ap())
            W3_t = pers.tile([H2, C_OUT], _F32)
            nc.sync.dma_start(out=W3_t[:], in_=W3_d.ap())
            b_ts = []
            for bd, bw in ((b1_d, H1), (b2_d, H2), (b3_d, C_OUT)):
                bt = pers.tile([P, bw], _F32, tag=f"b{bw}{bd.name}")
                nc.sync.dma_start(out=bt[:], in_=bd.ap())
                b_ts.append(bt)
            nself_t = pers.tile([P, NCOL], _F32)
            nc.sync.dma_start(out=nself_t[:], in_=nself_d.ap())

            h_cur = pers.tile([P, NCOL * SUB], _F32)   # shard features, padded to 8

            def transform_to_table(src_tile, F_l):
                """cc_in <- shard cols 0:SUB of src_tile; AllGather; expand."""
                nc.sync.dma_start(
                    out=cc_in.ap().rearrange("(t p) s -> p t s", p=P),
                    in_=src_tile[:].rearrange("p (t s) -> p t s", s=SUB))
                nc.gpsimd.collective_compute(
                    "AllGather", mybir.AluOpType.bypass,
                    ins=[cc_in.ap()], outs=[cc_out.ap()],
                    replica_groups=[list(range(N_CORES))])
                nc.sync.dma_start(
                    out=tab.ap()[0:NTOT // GROUP, 0:PAY].rearrange(
                        "g (n s) -> g n s", n=GROUP),
                    in_=cc_out.ap().rearrange("(g n) s -> g n s", n=GROUP))

            def conv_layer(li, F_l):
                """Gather+extract+reduce from `tab` into acc [P, NCOL*F_l]."""
                acc = accp.tile([P, NCOL * SUB], _F32, tag=f"acc{li}")
                nc.vector.memset(acc[:], 0.0)
                for wdw in range(n_win):
                    fb0 = wdw * FB_W
                    fbn = min(FB_W, fb_total - fb0)
                    calls = (fbn * P) // NI
                    gbuf = gat.tile([P, FB_W * PAY], _F32, tag="gbuf")
                    it = work.tile([P, WIN, NI // 16], _I16, tag="gidx")
                    nc.sync.dma_start(
                        out=it[:, 0:calls, :],
                        in_=idx_d.ap()[wdw * WIN:wdw * WIN + calls].rearrange(
                            "c p w -> p c w"))
                    for cix in range(calls):
                        call = wdw * WIN + cix
                        fb_c = cix * (NI // P)
                        _raw_dma_gather(
                            nc.gpsimd,
                            gbuf[:].rearrange("p (w y) -> p w y", y=PAY)[
                                :, fb_c:fb_c + NI // P, :],
                            tab.ap()[:, 0:PAY], it[:, cix, :], NI, PAY, TAB_STRIDE,
                            queue_num=call % N_QUEUES)
                    mks = work.tile([P, GROUP, FB_W], _F32, tag="mks")
                    nc.sync.dma_start(out=mks[:, :, 0:fbn],
                                      in_=mask_d.ap()[:, :, fb0:fb0 + fbn])
                    gv = gbuf[:].rearrange("p (w y) -> p w y", y=PAY)
                    msg = gat.tile([P, FB_W * SUB], _F32, tag="msgw")
                    mv = msg[:].rearrange("p (w f) -> p w f", f=F_l)[:, 0:fbn, :]
                    for g in range(GROUP):
                        mk0 = mks[:, g, 0:fbn]
                        mk = bass.AP(mk0.tensor, mk0.offset,
                                     [mk0.ap[0], mk0.ap[1], [0, F_l]])
                        src_g = gv[:, 0:fbn, g * SUB:g * SUB + F_l]
                        if g == 0:
                            nc.vector.tensor_tensor(out=mv, in0=src_g, in1=mk,
                                                    op=mybir.AluOpType.mult)
                        else:
                            t2 = gat.tile([P, FB_W * SUB], _F32, tag="t2")
                            t2v = t2[:].rearrange("p (w f) -> p w f", f=F_l)[
                                :, 0:fbn, :]
                            nc.vector.tensor_tensor(out=t2v, in0=src_g, in1=mk,
                                                    op=mybir.AluOpType.mult)
                            nc.vector.tensor_add(out=mv, in0=mv, in1=t2v)
                    # rounds intersecting this window: partial strided adds
                    accv = acc[:].rearrange("p (t s) -> p t s", s=SUB)
                    msgv = msg[:].rearrange("p (w f) -> p w f", f=F_l)
                    for r, n_r in enumerate(n_r_list):
                        ncols = (int(n_r) + P - 1) // P
                        rb = base_list[r] // P
                        lo = max(rb, fb0)
                        hi = min(rb + ncols, fb0 + fbn)
                        if lo >= hi:
                            continue
                        nc.vector.tensor_add(
                            out=accv[:, lo - rb:hi - rb, 0:F_l],
                            in0=accv[:, lo - rb:hi - rb, 0:F_l],
                            in1=msgv[:, lo - fb0:hi - fb0, :])
                return acc

            def add_self_bias_elu(acc, F_l, bias_t, last=False):
                """acc += nself*h_cur ; += bias ; elu (skip elu if last)."""
                accv = acc[:].rearrange("p (t s) -> p t s", s=SUB)[:, :, 0:F_l]
                hv = h_cur[:].rearrange("p (t s) -> p t s", s=SUB)[:, :, 0:F_l]
                nt = nself_t[:]
                nsv = bass.AP(nt.tensor, nt.offset,
                              [nt.ap[0], nt.ap[1], [0, F_l]])
                sc = work.tile([P, NCOL * SUB], _F32, tag="sc")
                scv = sc[:].rearrange("p (t s) -> p t s", s=SUB)[:, :, 0:F_l]
                nc.vector.tensor_tensor(out=scv, in0=hv, in1=nsv,
                                        op=mybir.AluOpType.mult)
                nc.vector.tensor_add(out=accv, in0=accv, in1=scv)
                bt0 = bias_t[:]
                bv = bass.AP(bt0.tensor, bt0.offset,
                             [bt0.ap[0], [0, NCOL], bt0.ap[1]])
                nc.vector.tensor_add(out=accv, in0=accv, in1=bv)
                if not last:
                    # elu(x) = relu(x) + exp(min(x,0)) - 1
                    t_neg = work.tile([P, NCOL * SUB], _F32, tag="t_neg")
                    tnv = t_neg[:].rearrange("p (t s) -> p t s", s=SUB)[:, :, 0:F_l]
                    nc.vector.tensor_scalar_min(tnv, accv, 0.0)
                    nc.scalar.activation(tnv, tnv,
                                         mybir.ActivationFunctionType.Exp)
                    nc.vector.tensor_scalar_max(accv, accv, 0.0)
                    nc.vector.tensor_add(out=accv, in0=accv, in1=tnv)
                    nc.vector.tensor_scalar_add(accv, accv, -1.0)

            def matmul_shard(dst_tile, src_tile, Wt, F_in_l, F_out_l):
                """dst[:, t*SUB ...] <- (src rows) @ W  via PE transpose."""
                for t in range(NCOL):
                    xT = psum.tile([P, P], _F32, tag="ps_t")
                    nc.tensor.transpose(
                        out=xT[0:F_in_l, :],
                        in_=src_tile[:].rearrange("p (c s) -> p c s", s=SUB)[
                            :, t, 0:F_in_l],
                        identity=ident[:])
                    xTs = work.tile([P, P], _F32, tag="xTs")
                    nc.vector.tensor_copy(out=xTs[0:F_in_l, :], in_=xT[0:F_in_l, :])
                    mp = psum.tile([P, 16], _F32, tag="ps_m")
                    nc.tensor.matmul(mp[:, 0:F_out_l], lhsT=xTs[0:F_in_l, :],
                                     rhs=Wt[:], start=True, stop=True)
                    nc.vector.tensor_copy(
                        out=dst_tile[:].rearrange("p (c s) -> p c s", s=SUB)[
                            :, t, 0:F_out_l],
                        in_=mp[:, 0:F_out_l])

            # ---------------- layer 1: m1 = x @ W1 (full 256-wide transform)
            nc.vector.memset(h_cur[:], 0.0)
            m1 = pers.tile([P, NCOL * SUB], _F32)
            nc.vector.memset(m1[:], 0.0)
            for t in range(NCOL):
                xt = work.tile([P, F_IN], _F32, tag="xt")
                nc.sync.dma_start(out=xt[:], in_=x_d.ap()[t * P:(t + 1) * P, :])
                m1p = psum.tile([P, H1], _F32, tag="ps_m1")
                for half in range(2):
                    xT = psum.tile([P, P], _F32, tag="ps_t")
                    nc.tensor.transpose(out=xT[:],
                                        in_=xt[:, half * P:(half + 1) * P],
                                        identity=ident[:])
                    xTs = work.tile([P, P], _F32, tag="xTs")
                    nc.vector.tensor_copy(out=xTs[:], in_=xT[:])
                    nc.tensor.matmul(m1p[:], lhsT=xTs[:],
                                     rhs=W1_t[:, half, :],
                                     start=(half == 0), stop=(half == 1))
                nc.vector.tensor_copy(
                    out=m1[:].rearrange("p (c s) -> p c s", s=SUB)[:, t, 0:H1],
                    in_=m1p[:])
            # x itself is h for the self-loop of layer 1?  No: layer1 self term
            # uses m1 (aggregation of m1 rows).  h_cur := m1 for self-contrib.
            nc.vector.tensor_copy(out=h_cur[:], in_=m1[:])
            transform_to_table(m1, H1)
            acc1 = conv_layer(0, H1)
            add_self_bias_elu(acc1, H1, b_ts[0])
            # h2 = acc1 (8 cols used)
            nc.vector.memset(h_cur[:], 0.0)
            nc.vector.tensor_copy(
                out=h_cur[:].rearrange("p (c s) -> p c s", s=SUB)[:, :, 0:H1],
                in_=acc1[:].rearrange("p (c s) -> p c s", s=SUB)[:, :, 0:H1])

            # ---------------- layer 2: m2 = h2 @ W2
            m2 = pers.tile([P, NCOL * SUB], _F32, tag="m2")
            nc.vector.memset(m2[:], 0.0)
            matmul_shard(m2, h_cur, W2_t, H1, H2)
            nc.vector.tensor_copy(out=h_cur[:], in_=m2[:])
            transform_to_table(m2, H2)
            acc2 = conv_layer(1, H2)
            add_self_bias_elu(acc2, H2, b_ts[1])
            nc.vector.memset(h_cur[:], 0.0)
            nc.vector.tensor_copy(
                out=h_cur[:].rearrange("p (c s) -> p c s", s=SUB)[:, :, 0:H2],
                in_=acc2[:].rearrange("p (c s) -> p c s", s=SUB)[:, :, 0:H2])

            # ---------------- layer 3: aggregate h3 first, transform after
            transform_to_table(h_cur, H2)
            acc3 = conv_layer(2, H2)
            # self term for aggregation of h3
            accv = acc3[:].rearrange("p (t s) -> p t s", s=SUB)[:, :, 0:H2]
            hv = h_cur[:].rearrange("p (t s) -> p t s", s=SUB)[:, :, 0:H2]
            nt = nself_t[:]
            nsv = bass.AP(nt.tensor, nt.offset, [nt.ap[0], nt.ap[1], [0, H2]])
            sc = work.tile([P, NCOL * SUB], _F32, tag="sc")
            scv = sc[:].rearrange("p (t s) -> p t s", s=SUB)[:, :, 0:H2]
            nc.vector.tensor_tensor(out=scv, in0=hv, in1=nsv,
                                    op=mybir.AluOpType.mult)
            nc.vector.tensor_add(out=accv, in0=accv, in1=scv)
            # out = agg3 @ W3 + b3
            outt = work.tile([P, NCOL * C_OUT], _F32, tag="outt")
            for t in range(NCOL):
                aT = psum.tile([P, P], _F32, tag="ps_t")
                nc.tensor.transpose(
                    out=aT[0:H2, :],
                    in_=acc3[:].rearrange("p (c s) -> p c s", s=SUB)[:, t, 0:H2],
                    identity=ident[:])
                aTs = work.tile([P, P], _F32, tag="xTs")
                nc.vector.tensor_copy(out=aTs[0:H2, :], in_=aT[0:H2, :])
                op = psum.tile([P, 16], _F32, tag="ps_m")
                nc.tensor.matmul(op[:, 0:C_OUT], lhsT=aTs[0:H2, :], rhs=W3_t[:],
                                 start=True, stop=True)
                bv = b_ts[2][:]
                nc.vector.tensor_add(
                    out=outt[:, t * C_OUT:(t + 1) * C_OUT],
                    in0=op[:, 0:C_OUT], in1=bv)
            nc.sync.dma_start(out=out_d.ap(), in_=outt[:])

    nc.compile()
    return nc


def kernel(x, edge_index, edge_weight, W1, b1, W2, b2, W3, b3):
    x = np.asarray(x, np.float32)
    cores, row_of, norm_self = _prep(np.asarray(edge_index),
                                     np.asarray(edge_weight))
    # all cores must share one program: pad structures to common sizes
    fb_total = max(c["fb_total"] for c in cores)
    fb_total = ((fb_total * P + NI - 1) // NI) * NI // P
    n_calls = fb_total * P // NI
    rmax = max(len(c["n_r"]) for c in cores)
    # common padded round bases: use per-core maxima so one program serves all
    n_r_com = np.zeros(rmax, np.int64)
    for c in cores:
        n_r_com[: len(c["n_r"])] = np.maximum(n_r_com[: len(c["n_r"])], c["n_r"])
    n_r_pad = ((n_r_com + P - 1) // P) * P
    base_com = np.zeros(rmax + 1, np.int64)
    np.cumsum(n_r_pad, out=base_com[1:])
    need_fb = int(base_com[-1]) // P
    fb_total = max(fb_total, ((need_fb * P + NI - 1) // NI) * NI // P)
    n_calls = fb_total * P // NI

    # re-layout each core's slots onto the COMMON round bases
    def relayout(k):
        c = cores[k]
        slots = fb_total * P
        grp = np.full(slots, NGRP - 1, np.int16)
        msk = np.zeros((GROUP, slots), np.float32)
        om = c["masks"].transpose(0, 2, 1).reshape(GROUP, -1)  # [g, oldslots]
        for r in range(len(c["n_r"])):
            ob, nb = int(c["base"][r]), int(base_com[r])
            ln = int(c["n_r_pad"][r])
            grp[nb:nb + ln] = np.frombuffer(
                c["gidx"], np.int16).reshape(-1)[0:0] if False else grp[nb:nb + ln]
        # simpler: rebuild from original slot arrays
        return grp, msk

    # Rebuild slot arrays directly on common bases (redo cheap part of prep)
    src = np.asarray(edge_index[0], np.int64)
    dst = np.asarray(edge_index[1], np.int64)
    w64 = np.asarray(edge_weight, np.float64)
    deg = np.zeros(N, np.float64)
    np.add.at(deg, dst, w64)
    deg += 1.0
    dis = 1.0 / np.sqrt(deg)
    norm_edge = (dis[src] * w64 * dis[dst]).astype(np.float32)
    core_of = dst // NPC

    gidx_all, masks_all, nself_all, xs = [], [], [], []
    for k in range(N_CORES):
        c = cores[k]
        sigma = c["sigma"]
        sel = np.nonzero(core_of == k)[0]
        rank_of_local = np.empty(NPC, np.int64)
        rank_of_local[sigma] = np.arange(NPC)
        rank = rank_of_local[dst[sel] - k * NPC]
        order = np.argsort(rank, kind="stable")
        sel = sel[order]
        rank = rank[order]
        degs_sorted = np.bincount(rank, minlength=NPC)
        starts = np.zeros(NPC + 1, np.int64)
        np.cumsum(degs_sorted, out=starts[1:])
        rnd = np.arange(len(sel)) - starts[rank]
        slots = fb_total * P
        grp = np.full(slots, NGRP - 1, np.int16)
        sel4 = np.zeros(slots, np.int64)
        nrm = np.zeros(slots, np.float32)
        j = base_com[rnd] + rank
        gr = row_of[src[sel]]
        grp[j] = (gr // GROUP).astype(np.int16)
        sel4[j] = gr % GROUP
        nrm[j] = norm_edge[sel]
        masks = np.zeros((GROUP, slots), np.float32)
        masks[sel4, np.arange(slots)] = nrm
        masks = masks.reshape(GROUP, fb_total, P).transpose(2, 0, 1)
        g16 = grp.reshape(n_calls, NI // 16, 16).transpose(0, 2, 1)
        gidx_all.append(np.ascontiguousarray(np.tile(g16, (1, 8, 1))))
        masks_all.append(np.ascontiguousarray(masks))
        ns = np.zeros(NROW, np.float32)
        ns[:NPC] = norm_self[k * NPC + sigma]
        nself_all.append(ns.reshape(NCOL, P).T.copy())
        xp = np.zeros((NROW, F_IN), np.float32)
        xp[:NPC] = x[k * NPC + sigma]
        xs.append(xp)

    key = (fb_total, n_calls, rmax)
    if key not in _CACHE:
        _CACHE[key] = _build(fb_total, n_calls,
                             [int(v) for v in n_r_com],
                             [int(v) for v in base_com])
    nc = _CACHE[key]

    ins = []
    for k in range(N_CORES):
        ins.append({
            "x": xs[k],
            "W1": np.asarray(W1, np.float32), "W2": np.asarray(W2, np.float32),
            "W3": np.asarray(W3, np.float32),
            "b1": np.tile(np.asarray(b1, np.float32).reshape(1, H1), (P, 1)),
            "b2": np.tile(np.asarray(b2, np.float32).reshape(1, H2), (P, 1)),
            "b3": np.tile(np.asarray(b3, np.float32).reshape(1, C_OUT), (P, 1)),
            "nself": nself_all[k],
            "gidx": gidx_all[k],
            "masks": masks_all[k],
        })
    trace = bool(os.environ.get("KERNEL_TRACE"))
    res = run_bass_kernel_spmd(nc, ins, core_ids=list(range(N_CORES)),
                               trace=trace)
    global LAST_EXEC_NS
    LAST_EXEC_NS = res.exec_time_ns
    out = np.empty((N, C_OUT), np.float32)
    for k in range(N_CORES):
        o = res.results[k]["out"]            # [P, NCOL*C_OUT]
        rows = o.reshape(P, NCOL, C_OUT).transpose(1, 0, 2).reshape(NROW, C_OUT)
        sigma = cores[k]["sigma"]
        out[k * NPC + sigma] = rows[:NPC]
    return out
